# revision 33
# baseline (speedup 1.0000x reference)
"""Extended Kalman Filter kernel for 8 Trainium2 NeuronCores.

Math: the EKF covariance recursion (P -> A P A^T + Q; S = C P C^T + R;
K = P C^T S^-1; P -> (I-KC)P) does not depend on the data, only on cov0.
When cov0 is identical across the batch (it is: broadcast 0.1*I), the
per-timestep Kalman gains K_t are batch-independent and can be
precomputed on the host. The device-side work collapses to a linear
time-varying recursion on the mean only:

    mean_{t+1} = M_t @ mean_t + N_t @ u_t + K_t @ z_t
    M_t = (I - K_t C) A,  N_t = (I - K_t C) Bm

Device mapping (pure data-parallel over batch, 4096 batch/core):
  * batch n in [0,4096) is split as n = p*32 + h*16 + q with p in [0,128)
    (SBUF partition of the batch-major staging tiles), h in {0,1} (the
    column-half / chain index), q in [0,16) (position in the 16-batch run).
    Each partition covers 32 consecutive batches -> 768B DMA runs.
  * On-chip state layout is "feature-major blocks": mean tile
    [96 = (q,i), 256 = (h,p)] so the recursion is a matmul with a
    block-diagonal stationary kron(I_16, M_t^T) of shape [96, 96].
  * u_t / z_t arrive batch-major (contiguous DMA), are transposed
    on the TensorEngine ([128, 96] / [128, 48] tiles -> PSUM), copied to
    SBUF, and injected with block-diagonal stationaries.
  * The updated mean (= the output for step t) is transposed back to
    batch-major on the TensorEngine and stored contiguously.
"""

import numpy as np

T, BFULL, D, O, U = 64, 32768, 6, 3, 6
NCORES = 8
BS = BFULL // NCORES      # 4096 batch per core
G = 16                    # batches per 6-row feature block (96 = G*D rows)
COLS = 256                # state columns = 2 halves * 128 partitions
KT = 8                    # timesteps per DMA staging group

_CACHE = {}
LAST_RESULTS = None       # BassKernelResults of the most recent device run
CHAIN_F32R = False        # set True for single-pass (reduced precision) chain


def _host_coeffs(cov0_row, A, Bm, Q_tril, C, R_tril):
    """Run the (batch-independent) covariance recursion on the host in
    float64; return per-step float32 coefficient matrices M_t, N_t, K_t."""
    A = np.asarray(A, np.float64)
    Bm = np.asarray(Bm, np.float64)
    Qt = np.asarray(Q_tril, np.float64)
    C = np.asarray(C, np.float64)
    Rt = np.asarray(R_tril, np.float64)
    Qc = Qt @ Qt.T
    Rc = Rt @ Rt.T
    P = np.asarray(cov0_row, np.float64)
    I = np.eye(D)
    Ms = np.empty((T, D, D), np.float32)
    Ns = np.empty((T, D, U), np.float32)
    Ks = np.empty((T, D, O), np.float32)
    for t in range(T):
        Pp = A @ P @ A.T + Qc
        S = C @ Pp @ C.T + Rc
        K = Pp @ C.T @ np.linalg.inv(S)
        IKC = I - K @ C
        Ms[t] = IKC @ A
        Ns[t] = IKC @ Bm
        Ks[t] = K
        P = IKC @ Pp
    return Ms, Ns, Ks


def _stationaries(Ms, Ns, Ks):
    """Block-diagonal lhsT stationaries, packed for the two combined
    matmuls. matmul computes lhsT.T @ rhs, so each diagonal block is the
    transpose of the coefficient matrix.

    rhs1 (combo1) rows = [mean (96) ; zT rows 0:32], lhsT1 [128, 96]
    rhs2 (combo2) rows = [zT rows 32:48 ; uT (96)], lhsT2 [112, 96]
    """
    SM = np.zeros((T, G * D, G * D), np.float32)
    SN = np.zeros((T, G * U, G * D), np.float32)
    SK = np.zeros((T, G * O, G * D), np.float32)
    for g in range(G):
        SM[:, g * D:(g + 1) * D, g * D:(g + 1) * D] = np.transpose(Ms, (0, 2, 1))
        SN[:, g * U:(g + 1) * U, g * D:(g + 1) * D] = np.transpose(Ns, (0, 2, 1))
        SK[:, g * O:(g + 1) * O, g * D:(g + 1) * D] = np.transpose(Ks, (0, 2, 1))
    S1 = np.concatenate([SM, SK[:, 0:32, :]], axis=1)          # [T, 128, 96]
    S2 = np.concatenate([SN, SK[:, 32:48, :]], axis=1)         # [T, 112, 96]
    # k-major so the device-side load is fully contiguous per partition
    S1 = np.ascontiguousarray(S1.transpose(1, 0, 2)).reshape(128, T * G * D)
    S2 = np.ascontiguousarray(S2.transpose(1, 0, 2)).reshape(112, T * G * D)
    return S1, S2


def _build_program():
    """Build (once) the Bass/Tile program shared by all 8 cores."""
    if "nc" in _CACHE:
        return _CACHE["nc"]

    import concourse.bacc as bacc
    import concourse.tile as tile
    from concourse import mybir

    f32 = mybir.dt.float32
    # chain-matmul operand dtype: float32r is single-pass on the PE but
    # rounds operands (TF32-like); float32 is exact via the LOW_HIGH
    # double-pass. The grading gate is fp32-envelope, so default to f32.
    cdt = mybir.dt.float32r if CHAIN_F32R else f32
    nc = bacc.Bacc("TRN2", target_bir_lowering=False, debug=False,
                   num_devices=NCORES)

    meas = nc.dram_tensor("meas", [T, BS, O], f32, kind="ExternalInput").ap()
    useq = nc.dram_tensor("useq", [T, BS, U], f32, kind="ExternalInput").ap()
    mean0 = nc.dram_tensor("mean0", [BS, D], f32, kind="ExternalInput").ap()
    stat1 = nc.dram_tensor("stat1", [128, T * G * D], cdt, kind="ExternalInput").ap()
    stat2 = nc.dram_tensor("stat2", [112, T * G * D], cdt, kind="ExternalInput").ap()
    ident = nc.dram_tensor("ident", [128, 128], f32, kind="ExternalInput").ap()
    # output stays in the on-chip feature-major layout [96=(q,i), 256=(h,p)];
    # the host permutes axes during the gather/unshard step.
    out = nc.dram_tensor("out", [T, G * D, COLS], cdt, kind="ExternalOutput").ap()

    RD = G * D   # 96 state rows
    RZ = G * O   # 48 z rows
    NG = T // KT

    with tile.TileContext(nc) as tc:
        with (
            tc.tile_pool(name="const", bufs=1) as const,
            tc.tile_pool(name="stage", bufs=2) as stage,
            tc.tile_pool(name="fm", bufs=16) as fm,
            tc.tile_pool(name="ps_u", bufs=3, space="PSUM") as ps_up,
            tc.tile_pool(name="ps_z", bufs=2, space="PSUM") as ps_zp,
            tc.tile_pool(name="ps_s", bufs=2, space="PSUM") as ps_sp,
        ):
            id_t = const.tile([128, 128], f32)
            nc.sync.dma_start(id_t[:], ident[:])
            s1_t = const.tile([128, T * RD], cdt)
            s2_t = const.tile([112, T * RD], cdt)

            def load_stats(g):
                fs = slice(g * KT * RD, (g + 1) * KT * RD)
                nc.sync.dma_start(s1_t[:, fs], stat1[:, fs])
                nc.sync.dma_start(s2_t[:, fs], stat2[:, fs])

            # The two 128-column halves are independent batches -> two
            # independent chains (h = 0, 1) interleaved on the PE so the
            # serial state->copy->state latency of one chain hides under
            # the other chain's matmuls.
            # combo1(t,h) = [mean_t (96 rows) ; zT_t rows 0:32]  -> lhsT stat1
            # combo2(t,h) = [uT_t (96 rows) ; zT_t rows 32:48]   -> lhsT stat2
            def alloc_combos(gidx):
                c1 = [fm.tile([128, COLS], cdt, tag="c1",
                              name=f"c1_{gidx}_{i}") for i in range(KT)]
                c2 = [fm.tile([112, COLS], cdt, tag="c2",
                              name=f"c2_{gidx}_{i}") for i in range(KT)]
                return c1, c2

            combo1, combo2 = alloc_combos(0)

            # initial state: load mean0 batch-major, transpose into combo1[0]
            m0 = stage.tile([128, 2 * RD], f32, tag="m0")
            nc.sync.dma_start(
                m0[:].rearrange("p (h f) -> p h f", h=2),
                mean0.rearrange("(p h q) i -> p h (q i)", h=2, p=128, q=G))
            ps0 = ps_up.tile([RD, COLS], f32, tag="ps_u")
            for h in range(2):
                nc.tensor.transpose(ps0[:, h * 128:(h + 1) * 128],
                                    m0[:, h * RD:(h + 1) * RD], id_t[:])
            nc.scalar.copy(combo1[0][0:RD, :], ps0[:])

            u_sts, z_sts, o_sts = {}, {}, {}

            def load_group(g):
                u_st = stage.tile([128, KT * 2 * RD], f32, tag="u_st",
                                  name=f"u_st_{g}", bufs=3)
                nc.sync.dma_start(
                    u_st[:].rearrange("p (t h f) -> p t h f", t=KT, h=2),
                    useq[g * KT:(g + 1) * KT].rearrange(
                        "t (p h q) u -> p t h (q u)", h=2, p=128, q=G))
                z_st = stage.tile([128, KT * 2 * RZ], f32, tag="z_st",
                                  name=f"z_st_{g}", bufs=3)
                nc.sync.dma_start(
                    z_st[:].rearrange("p (t h f) -> p t h f", t=KT, h=2),
                    meas[g * KT:(g + 1) * KT].rearrange(
                        "t (p h q) o -> p t h (q o)", h=2, p=128, q=G))
                u_sts[g], z_sts[g] = u_st, z_st

            def transpose_step(t, c1, c2):
                """PE transposes + copies filling combo tile pairs for step t."""
                g, tl = t // KT, t % KT
                ps_u = ps_up.tile([RD, COLS], f32, tag="ps_u")
                for h in range(2):
                    nc.tensor.transpose(
                        ps_u[:, h * 128:(h + 1) * 128],
                        u_sts[g][:, (tl * 2 + h) * RD:(tl * 2 + h + 1) * RD],
                        id_t[:])
                ps_z = ps_zp.tile([RZ, COLS], f32, tag="ps_z")
                for h in range(2):
                    nc.tensor.transpose(
                        ps_z[:, h * 128:(h + 1) * 128],
                        z_sts[g][:, (tl * 2 + h) * RZ:(tl * 2 + h + 1) * RZ],
                        id_t[:])
                nc.scalar.copy(c2[0:RD, :], ps_u[:])
                nc.vector.tensor_copy(c1[RD:128, :], ps_z[0:32, :])
                nc.vector.tensor_copy(c2[RD:112, :], ps_z[32:48, :])

            # prologue: group 0 (and its transposes); prefetch group 1.
            # Input loads go before the stats so the first transposes and
            # chain steps are not queued behind 5.9MB of stationaries.
            load_group(0)
            load_stats(0)
            load_group(1)
            load_stats(1)
            for tl in range(KT):
                transpose_step(tl, combo1[tl], combo2[tl])

            for g in range(NG):
                if g + 2 < NG:
                    load_group(g + 2)
                    load_stats(g + 2)
                combo1_next, combo2_next = alloc_combos(g + 1)
                for tl in range(KT):
                    t = g * KT + tl
                    c1n = combo1_next[0] if tl == KT - 1 else combo1[tl + 1]
                    # chain matmuls in float32r: single-pass fp32 on the PE
                    # (vs LOW_HIGH double-pass) when the moving dim is >=256
                    ts = slice(t * RD, (t + 1) * RD)
                    ps_s = ps_sp.tile([RD, COLS], f32, tag="ps_s", bufs=1,
                                      name=f"ps_s_{t}")
                    nc.tensor.matmul(ps_s[:], s1_t[:, ts], combo1[tl][:],
                                     start=True, stop=False)
                    nc.tensor.matmul(ps_s[:], s2_t[:, ts], combo2[tl][:],
                                     start=False, stop=True)
                    nc.scalar.copy(c1n[0:RD, 0:128], ps_s[:, 0:128])
                    nc.vector.tensor_copy(c1n[0:RD, 128:COLS], ps_s[:, 128:COLS])
                    # the new mean IS the step-t output: store it directly.
                    # SWDGE (gpsimd) queue: the HWDGE/sync sequencer is busy
                    # with the input loads.
                    nc.scalar.dma_start(out[t], c1n[0:RD, :])
                    # fill PE pipeline while the state copies are in flight:
                    if g + 1 < NG:
                        transpose_step((g + 1) * KT + tl,
                                       combo1_next[tl], combo2_next[tl])
                combo1, combo2 = combo1_next, combo2_next

    nc.compile()
    _CACHE["nc"] = nc
    return nc


def _run_device(meas_np, useq_np, mean0_np, S1, S2, trace=False):
    global LAST_RESULTS
    from concourse import bass_utils

    nc = _build_program()
    ident = np.eye(128, dtype=np.float32)
    in_maps = []
    for m in range(NCORES):
        sl = slice(m * BS, (m + 1) * BS)
        in_maps.append({
            "meas": np.ascontiguousarray(meas_np[:, sl]),
            "useq": np.ascontiguousarray(useq_np[:, sl]),
            "mean0": np.ascontiguousarray(mean0_np[sl]),
            "stat1": S1, "stat2": S2, "ident": ident,
        })
    res = bass_utils.run_bass_kernel_spmd(
        nc, in_maps, core_ids=list(range(NCORES)), trace=trace)
    LAST_RESULTS = res
    # device output is feature-major [T, (q,i), (h,p)]; permute back to
    # batch-major (T, BS, D) with n = p*32 + h*16 + q per core, then concat
    outs = []
    for m in range(NCORES):
        o = res.results[m]["out"].reshape(T, G, D, 2, 128)
        outs.append(np.ascontiguousarray(
            o.transpose(0, 4, 3, 1, 2)).reshape(T, BS, D))
    return np.concatenate(outs, axis=1)


def _numpy_fallback(measurements, inputs_seq, mean0, cov0, A, Bm, Q_tril, C, R_tril):
    """General (per-batch covariance) EKF in vectorized numpy. Correctness
    fallback only; used when cov0 is not batch-uniform."""
    f = np.float32
    A = np.asarray(A, f); Bm = np.asarray(Bm, f); C = np.asarray(C, f)
    Qc = (np.asarray(Q_tril, f) @ np.asarray(Q_tril, f).T).astype(f)
    Rc = (np.asarray(R_tril, f) @ np.asarray(R_tril, f).T).astype(f)
    mean = np.asarray(mean0, f).copy()
    cov = np.asarray(cov0, f).copy()
    I = np.eye(D, dtype=f)
    outs = np.empty((T, mean.shape[0], D), f)
    for t in range(T):
        z = np.asarray(measurements[t], f)
        u = np.asarray(inputs_seq[t], f)
        pm = mean @ A.T + u @ Bm.T
        pc = np.einsum('ij,bjk,lk->bil', A, cov, A) + Qc
        innov = z - pm @ C.T
        S = np.einsum('ij,bjk,lk->bil', C, pc, C) + Rc
        PCt = np.einsum('bij,kj->bik', pc, C)
        K = PCt @ np.linalg.inv(S)
        mean = pm + np.einsum('bij,bj->bi', K, innov)
        cov = (I - np.einsum('bij,jk->bik', K, C)) @ pc
        outs[t] = mean
    return outs


def kernel(measurements, inputs_seq, mean0, cov0, A, Bm, Q_tril, C, R_tril):
    measurements = np.asarray(measurements)
    inputs_seq = np.asarray(inputs_seq)
    mean0 = np.asarray(mean0)
    cov0 = np.asarray(cov0)

    if np.ptp(cov0, axis=0).max() != 0.0:
        return _numpy_fallback(measurements, inputs_seq, mean0, cov0,
                               A, Bm, Q_tril, C, R_tril)

    Ms, Ns, Ks = _host_coeffs(cov0[0], A, Bm, Q_tril, C, R_tril)
    S1, S2 = _stationaries(Ms, Ns, Ks)
    return _run_device(measurements.astype(np.float32),
                       inputs_seq.astype(np.float32),
                       mean0.astype(np.float32), S1, S2,
                       trace=False)


# revision 34
# speedup vs baseline: 1.4380x; 1.4380x over previous
"""Extended Kalman Filter kernel for 8 Trainium2 NeuronCores.

Math: the EKF covariance recursion (P -> A P A^T + Q; S = C P C^T + R;
K = P C^T S^-1; P -> (I-KC)P) does not depend on the data, only on cov0.
When cov0 is identical across the batch (it is: broadcast 0.1*I), the
per-timestep Kalman gains K_t are batch-independent and can be
precomputed on the host. The device-side work collapses to a linear
time-varying recursion on the mean only:

    mean_{t+1} = M_t @ mean_t + N_t @ u_t + K_t @ z_t
    M_t = (I - K_t C) A,  N_t = (I - K_t C) Bm

Device mapping (pure data-parallel over batch, 4096 batch/core):
  * batch n in [0,4096) is split as n = p*32 + h*16 + q with p in [0,128)
    (SBUF partition of the batch-major staging tiles), h in {0,1} (the
    column-half / chain index), q in [0,16) (position in the 16-batch run).
    Each partition covers 32 consecutive batches -> 768B DMA runs.
  * On-chip state layout is "feature-major blocks": per half h a mean tile
    [96 = (q,i), 128 = p] so the recursion is a matmul with a
    block-diagonal stationary kron(I_16, M_t^T).
  * The two halves are independent batches -> two independent chains
    interleaved on the (in-order) PE queue, so one chain's serial
    matmul -> PSUM -> copy -> matmul latency hides under the other
    chain's matmuls and under the input/output transposes.
  * u_t / z_t arrive batch-major (contiguous DMA), are transposed on the
    TensorEngine ([128, 96] / [128, 48] tiles -> PSUM), copied to SBUF,
    and injected with block-diagonal stationaries; combo row packing
    lets one [128,96] + one [112,96] stationary cover mean/u/z at once.
  * The updated mean (= the output for step t) is transposed back to
    batch-major on the TensorEngine and stored contiguously per group.
"""

import numpy as np

T, BFULL, D, O, U = 64, 32768, 6, 3, 6
NCORES = 8
BS = BFULL // NCORES      # 4096 batch per core
G = 16                    # batches per 6-row feature block (96 = G*D rows)
COLS = 256                # 2 halves * 128 partitions
KT = 8                    # timesteps per DMA staging group

_CACHE = {}
LAST_RESULTS = None       # BassKernelResults of the most recent device run
CHAIN_F32R = False        # True: single-pass (reduced precision) chain matmuls


def _host_coeffs(cov0_row, A, Bm, Q_tril, C, R_tril):
    """Run the (batch-independent) covariance recursion on the host in
    float64; return per-step float32 coefficient matrices M_t, N_t, K_t."""
    A = np.asarray(A, np.float64)
    Bm = np.asarray(Bm, np.float64)
    Qt = np.asarray(Q_tril, np.float64)
    C = np.asarray(C, np.float64)
    Rt = np.asarray(R_tril, np.float64)
    Qc = Qt @ Qt.T
    Rc = Rt @ Rt.T
    P = np.asarray(cov0_row, np.float64)
    I = np.eye(D)
    Ms = np.empty((T, D, D), np.float32)
    Ns = np.empty((T, D, U), np.float32)
    Ks = np.empty((T, D, O), np.float32)
    for t in range(T):
        Pp = A @ P @ A.T + Qc
        S = C @ Pp @ C.T + Rc
        K = Pp @ C.T @ np.linalg.inv(S)
        IKC = I - K @ C
        Ms[t] = IKC @ A
        Ns[t] = IKC @ Bm
        Ks[t] = K
        P = IKC @ Pp
    return Ms, Ns, Ks


def _stationaries(Ms, Ns, Ks):
    """Block-diagonal lhsT stationaries, packed for the two combined
    matmuls. matmul computes lhsT.T @ rhs, so each diagonal block is the
    transpose of the coefficient matrix.

    rhs1 (combo1) rows = [mean (96) ; zT rows 0:32], lhsT1 [128, 96]
    rhs2 (combo2) rows = [uT (96) ; zT rows 32:48], lhsT2 [112, 96]
    """
    SM = np.zeros((T, G * D, G * D), np.float32)
    SN = np.zeros((T, G * U, G * D), np.float32)
    SK = np.zeros((T, G * O, G * D), np.float32)
    for g in range(G):
        SM[:, g * D:(g + 1) * D, g * D:(g + 1) * D] = np.transpose(Ms, (0, 2, 1))
        SN[:, g * U:(g + 1) * U, g * D:(g + 1) * D] = np.transpose(Ns, (0, 2, 1))
        SK[:, g * O:(g + 1) * O, g * D:(g + 1) * D] = np.transpose(Ks, (0, 2, 1))
    S1 = np.concatenate([SM, SK[:, 0:32, :]], axis=1)          # [T, 128, 96]
    S2 = np.concatenate([SN, SK[:, 32:48, :]], axis=1)         # [T, 112, 96]
    # k-major so the device-side load is fully contiguous per partition
    S1 = np.ascontiguousarray(S1.transpose(1, 0, 2)).reshape(128, T * G * D)
    S2 = np.ascontiguousarray(S2.transpose(1, 0, 2)).reshape(112, T * G * D)
    return S1, S2


def _build_program():
    """Build (once) the Bass/Tile program shared by all 8 cores."""
    if "nc" in _CACHE:
        return _CACHE["nc"]

    import concourse.bacc as bacc
    import concourse.tile as tile
    from concourse import mybir

    f32 = mybir.dt.float32
    # chain-matmul operand dtype: float32r is single-pass on the PE but
    # rounds operands (TF32-like); float32 is exact via the LOW_HIGH
    # double-pass. The grading gate is fp32-envelope, so default to f32.
    cdt = mybir.dt.float32r if CHAIN_F32R else f32
    nc = bacc.Bacc("TRN2", target_bir_lowering=False, debug=False,
                   num_devices=NCORES)

    meas = nc.dram_tensor("meas", [T, BS, O], f32, kind="ExternalInput").ap()
    useq = nc.dram_tensor("useq", [T, BS, U], f32, kind="ExternalInput").ap()
    mean0 = nc.dram_tensor("mean0", [BS, D], f32, kind="ExternalInput").ap()
    stat1 = nc.dram_tensor("stat1", [128, T * G * D], cdt, kind="ExternalInput").ap()
    stat2 = nc.dram_tensor("stat2", [112, T * G * D], cdt, kind="ExternalInput").ap()
    ident = nc.dram_tensor("ident", [128, 128], f32, kind="ExternalInput").ap()
    identr = nc.dram_tensor("identr", [96, 96], cdt, kind="ExternalInput").ap()
    out = nc.dram_tensor("out", [T, BS, D], f32, kind="ExternalOutput").ap()

    RD = G * D   # 96 state rows
    RZ = G * O   # 48 z rows
    NG = T // KT

    with tile.TileContext(nc) as tc:
        with (
            tc.tile_pool(name="const", bufs=1) as const,
            tc.tile_pool(name="stage", bufs=2) as stage,
            tc.tile_pool(name="fm", bufs=16) as fm,
            tc.tile_pool(name="ps_u", bufs=2, space="PSUM") as ps_up,
            tc.tile_pool(name="ps_z", bufs=2, space="PSUM") as ps_zp,
            tc.tile_pool(name="ps_s", bufs=1, space="PSUM") as ps_sp,
            tc.tile_pool(name="ps_o", bufs=2, space="PSUM") as ps_op,
        ):
            id_t = const.tile([128, 128], f32)
            nc.sync.dma_start(id_t[:], ident[:])
            idr_t = const.tile([96, 96], cdt)
            nc.sync.dma_start(idr_t[:], identr[:])
            s1_t = const.tile([128, T * RD], cdt)
            s2_t = const.tile([112, T * RD], cdt)

            def load_stats(g):
                fs = slice(g * KT * RD, (g + 1) * KT * RD)
                nc.sync.dma_start(s1_t[:, fs], stat1[:, fs])
                nc.sync.dma_start(s2_t[:, fs], stat2[:, fs])

            # per-half combo tiles (h = chain index):
            # combo1(t,h) = [mean_t (96 rows) ; zT_t rows 0:32]  -> lhsT stat1
            # combo2(t,h) = [uT_t (96 rows) ; zT_t rows 32:48]   -> lhsT stat2
            def alloc_combos(gidx):
                c1 = [[fm.tile([128, 128], cdt, tag=f"c1h{h}",
                               name=f"c1_{gidx}_{i}_{h}") for h in range(2)]
                      for i in range(KT)]
                c2 = [[fm.tile([112, 128], cdt, tag=f"c2h{h}",
                               name=f"c2_{gidx}_{i}_{h}") for h in range(2)]
                      for i in range(KT)]
                return c1, c2

            combo1, combo2 = alloc_combos(0)

            # initial state: load mean0 batch-major, transpose into combo1[0]
            m0 = stage.tile([128, 2 * RD], f32, tag="m0")
            nc.sync.dma_start(
                m0[:].rearrange("p (h f) -> p h f", h=2),
                mean0.rearrange("(p h q) i -> p h (q i)", h=2, p=128, q=G))
            ps0 = ps_up.tile([RD, COLS], f32, tag="ps_u")
            for h in range(2):
                nc.tensor.transpose(ps0[:, h * 128:(h + 1) * 128],
                                    m0[:, h * RD:(h + 1) * RD], id_t[:])
            nc.scalar.copy(combo1[0][0][0:RD, :], ps0[:, 0:128])
            nc.scalar.copy(combo1[0][1][0:RD, :], ps0[:, 128:COLS])

            u_sts, z_sts, o_sts = {}, {}, {}

            def load_group(g):
                u_st = stage.tile([128, KT * 2 * RD], f32, tag="u_st",
                                  name=f"u_st_{g}", bufs=3)
                nc.sync.dma_start(
                    u_st[:].rearrange("p (t h f) -> p t h f", t=KT, h=2),
                    useq[g * KT:(g + 1) * KT].rearrange(
                        "t (p h q) u -> p t h (q u)", h=2, p=128, q=G))
                z_st = stage.tile([128, KT * 2 * RZ], f32, tag="z_st",
                                  name=f"z_st_{g}", bufs=3)
                nc.sync.dma_start(
                    z_st[:].rearrange("p (t h f) -> p t h f", t=KT, h=2),
                    meas[g * KT:(g + 1) * KT].rearrange(
                        "t (p h q) o -> p t h (q o)", h=2, p=128, q=G))
                u_sts[g], z_sts[g] = u_st, z_st

            def transpose_step(t, c1, c2):
                """PE transposes + copies filling combo tile pairs for step t."""
                g, tl = t // KT, t % KT
                ps_u = ps_up.tile([RD, COLS], f32, tag="ps_u")
                for h in range(2):
                    nc.tensor.transpose(
                        ps_u[:, h * 128:(h + 1) * 128],
                        u_sts[g][:, (tl * 2 + h) * RD:(tl * 2 + h + 1) * RD],
                        id_t[:])
                ps_z = ps_zp.tile([RZ, COLS], f32, tag="ps_z")
                for h in range(2):
                    nc.tensor.transpose(
                        ps_z[:, h * 128:(h + 1) * 128],
                        z_sts[g][:, (tl * 2 + h) * RZ:(tl * 2 + h + 1) * RZ],
                        id_t[:])
                for h in range(2):
                    cs = slice(h * 128, (h + 1) * 128)
                    nc.scalar.copy(c2[h][0:RD, :], ps_u[:, cs])
                    nc.vector.tensor_copy(c1[h][RD:128, :], ps_z[0:32, cs])
                    nc.vector.tensor_copy(c2[h][RD:112, :], ps_z[32:48, cs])

            def out_transpose(t, c1_next):
                """Transpose mean_{t+1} (= output t) to batch-major."""
                g, tl = t // KT, t % KT
                ps_o = ps_op.tile([128, 2 * RD], cdt, tag="ps_o")
                for h in range(2):
                    nc.tensor.transpose(
                        ps_o[:, h * RD:(h + 1) * RD],
                        c1_next[h][0:RD, :],
                        idr_t[:])
                nc.scalar.copy(
                    o_sts[g][:, tl * 2 * RD:(tl + 1) * 2 * RD], ps_o[:])

            # prologue: group 0 (and its transposes); prefetch group 1.
            # Input loads go before the stats so the first transposes and
            # chain steps are not queued behind 5.9MB of stationaries.
            load_group(0)
            load_stats(0)
            load_group(1)
            load_stats(1)
            for tl in range(KT):
                transpose_step(tl, combo1[tl], combo2[tl])

            for g in range(NG):
                o_sts[g] = stage.tile([128, KT * 2 * RD], f32, tag="o_st",
                                      name=f"o_st_{g}")
                if g + 2 < NG:
                    load_group(g + 2)
                    load_stats(g + 2)
                combo1_next, combo2_next = alloc_combos(g + 1)
                for tl in range(KT):
                    t = g * KT + tl
                    c1n = combo1_next[0] if tl == KT - 1 else combo1[tl + 1]
                    ts = slice(t * RD, (t + 1) * RD)
                    # two independent chains, interleaved on the PE
                    for h in range(2):
                        ps_s = ps_sp.tile([RD, 128], f32, tag=f"ps_s{h}",
                                          name=f"ps_s_{t}_{h}", bufs=1)
                        nc.tensor.matmul(ps_s[:], s1_t[:, ts],
                                         combo1[tl][h][:], start=True, stop=False)
                        nc.tensor.matmul(ps_s[:], s2_t[:, ts],
                                         combo2[tl][h][:], start=False, stop=True)
                        if h == 0:
                            nc.scalar.copy(c1n[h][0:RD, :], ps_s[:])
                        else:
                            nc.vector.tensor_copy(c1n[h][0:RD, :], ps_s[:])
                    # fill PE pipeline while the state copies are in flight:
                    if t > 0:
                        # mean_t (= output t-1) lives in combo1[tl][h][0:96]
                        out_transpose(t - 1, combo1[tl])
                    if g + 1 < NG:
                        transpose_step((g + 1) * KT + tl,
                                       combo1_next[tl], combo2_next[tl])
                    if tl == 0 and g > 0:
                        nc.sync.dma_start(
                            out[(g - 1) * KT:g * KT].rearrange(
                                "t (p h q) i -> p t h (q i)", h=2, p=128, q=G),
                            o_sts[g - 1][:].rearrange(
                                "p (t h f) -> p t h f", t=KT, h=2))
                combo1, combo2 = combo1_next, combo2_next

            # epilogue: final output transpose + last group store
            out_transpose(T - 1, combo1[0])
            nc.sync.dma_start(
                out[(NG - 1) * KT:].rearrange(
                    "t (p h q) i -> p t h (q i)", h=2, p=128, q=G),
                o_sts[NG - 1][:].rearrange("p (t h f) -> p t h f", t=KT, h=2))

    nc.compile()
    _CACHE["nc"] = nc
    return nc


def _run_device(meas_np, useq_np, mean0_np, S1, S2, trace=False):
    global LAST_RESULTS
    from concourse import bass_utils

    nc = _build_program()
    ident = np.eye(128, dtype=np.float32)
    identr = np.eye(96, dtype=np.float32)
    in_maps = []
    for m in range(NCORES):
        sl = slice(m * BS, (m + 1) * BS)
        in_maps.append({
            "meas": np.ascontiguousarray(meas_np[:, sl]),
            "useq": np.ascontiguousarray(useq_np[:, sl]),
            "mean0": np.ascontiguousarray(mean0_np[sl]),
            "stat1": S1, "stat2": S2, "ident": ident, "identr": identr,
        })
    res = bass_utils.run_bass_kernel_spmd(
        nc, in_maps, core_ids=list(range(NCORES)), trace=trace)
    LAST_RESULTS = res
    return np.concatenate([res.results[m]["out"] for m in range(NCORES)], axis=1)


def _numpy_fallback(measurements, inputs_seq, mean0, cov0, A, Bm, Q_tril, C, R_tril):
    """General (per-batch covariance) EKF in vectorized numpy. Correctness
    fallback only; used when cov0 is not batch-uniform."""
    f = np.float32
    A = np.asarray(A, f); Bm = np.asarray(Bm, f); C = np.asarray(C, f)
    Qc = (np.asarray(Q_tril, f) @ np.asarray(Q_tril, f).T).astype(f)
    Rc = (np.asarray(R_tril, f) @ np.asarray(R_tril, f).T).astype(f)
    mean = np.asarray(mean0, f).copy()
    cov = np.asarray(cov0, f).copy()
    I = np.eye(D, dtype=f)
    outs = np.empty((T, mean.shape[0], D), f)
    for t in range(T):
        z = np.asarray(measurements[t], f)
        u = np.asarray(inputs_seq[t], f)
        pm = mean @ A.T + u @ Bm.T
        pc = np.einsum('ij,bjk,lk->bil', A, cov, A) + Qc
        innov = z - pm @ C.T
        S = np.einsum('ij,bjk,lk->bil', C, pc, C) + Rc
        PCt = np.einsum('bij,kj->bik', pc, C)
        K = PCt @ np.linalg.inv(S)
        mean = pm + np.einsum('bij,bj->bi', K, innov)
        cov = (I - np.einsum('bij,jk->bik', K, C)) @ pc
        outs[t] = mean
    return outs


def kernel(measurements, inputs_seq, mean0, cov0, A, Bm, Q_tril, C, R_tril):
    measurements = np.asarray(measurements)
    inputs_seq = np.asarray(inputs_seq)
    mean0 = np.asarray(mean0)
    cov0 = np.asarray(cov0)

    if np.ptp(cov0, axis=0).max() != 0.0:
        return _numpy_fallback(measurements, inputs_seq, mean0, cov0,
                               A, Bm, Q_tril, C, R_tril)

    Ms, Ns, Ks = _host_coeffs(cov0[0], A, Bm, Q_tril, C, R_tril)
    S1, S2 = _stationaries(Ms, Ns, Ks)
    return _run_device(measurements.astype(np.float32),
                       inputs_seq.astype(np.float32),
                       mean0.astype(np.float32), S1, S2,
                       trace=False)
